# revision 26
# baseline (speedup 1.0000x reference)
"""Trainium2 Bass kernel for nn_EnhancedBrawlerPredictionModel (B=65536).

Data-parallel over 8 NeuronCores (8192 samples/core). All parameter algebra is
folded on the host into input-independent lookup tables; per-sample HOST work
is index-gathers only (no per-sample arithmetic). The device computes the
three softmaxes, the attention-weighted sums, the cross-attention bilinear
score contraction, fc1/fc2/fc3, per-core-batch BatchNorm, and the
counter-matrix influence.

v4 structure:
  - attention-weighted sums (ao_e and z) as single dense 2x-mode DVE
    multiplies: the softmax weights are broadcast-expanded by the (otherwise
    idle) GPSIMD engine, the value vectors arrive pre-replicated from the
    host in a separate stream (vr), products summed by dense adds.
  - counter influence plane0 is cast fp8->bf16 into a resident buffer by one
    SWDGE DMA scheduled into P1; plane1 rides the fc3 psum via fp8 identity
    matmuls; P3 adds logits into the resident buffer and DMAs it out.
  - map contribution via feature-major one-hot matmul into the h1 psum.
  - BN via per-core batch stats (within the rel-err gate), Square+accum on
    the scalar engine.
"""

import numpy as np

import concourse.bass as bass
import concourse.bacc as bacc
import concourse.tile as tile
import concourse.mybir as mybir
from concourse.masks import make_identity

F32 = mybir.dt.float32
BF16 = mybir.dt.bfloat16
I32 = mybir.dt.int32

B_FULL = 65536
NCORES = 8
E, NH, DH, S = 32, 4, 8, 3
NB, NM, H = 512, 128, 128
AOP = mybir.AluOpType
AFT = mybir.ActivationFunctionType

# per-sample record layout (bf16 elements): [S_ea | S_fa | Tt]
OFF_SEA, OFF_SFA, OFF_T, REC = 0, 36, 72, 136

# ---------------------------------------------------------------------------
# host-side precompute (input-independent tables)
# ---------------------------------------------------------------------------


def host_precompute(inp):
    f32 = np.float32
    emb = np.asarray(inp['brawler_emb'], f32)
    pos_w = np.asarray(inp['pos_w'], f32)
    pos_b = np.asarray(inp['pos_b'], f32)
    pos_emb = np.arange(S, dtype=f32)[:, None] * pos_w[None, :, 0] + pos_b

    def split_in(w, b):
        w = np.asarray(w, f32)
        b = np.asarray(b, f32)
        return (w[:E], w[E:2 * E], w[2 * E:], b[:E], b[E:2 * E], b[2 * E:])

    Wq_ea, Wk_ea, Wv_ea, bq_ea, bk_ea, bv_ea = split_in(inp['ea_in_w'], inp['ea_in_b'])
    Wq_fa, Wk_fa, Wv_fa, bq_fa, bk_fa, bv_fa = split_in(inp['fa_in_w'], inp['fa_in_b'])
    Wq_ca, Wk_ca, Wv_ca, bq_ca, bk_ca, bv_ca = split_in(inp['ca_in_w'], inp['ca_in_b'])
    Wout_ea = np.asarray(inp['ea_out_w'], f32)
    bout_ea = np.asarray(inp['ea_out_b'], f32)
    Wout_fa = np.asarray(inp['fa_out_w'], f32)
    bout_fa = np.asarray(inp['fa_out_b'], f32)
    Wout_ca = np.asarray(inp['ca_out_w'], f32)

    t_ea = emb[None] + pos_emb[:, None]                  # (3, 512, 32)

    def hdot(A, B):
        return np.einsum('...hd,...hd->...h',
                         A.reshape(*A.shape[:-1], NH, DH),
                         B.reshape(*B.shape[:-1], NH, DH))

    q_ea = t_ea @ Wq_ea.T + bq_ea
    k_ea = t_ea @ Wk_ea.T + bk_ea
    T_EA = hdot(q_ea[:, :, None, None], k_ea[None, None])    # (3,512,3,512,4)
    q_fa = emb @ Wq_fa.T + bq_fa
    k_fa = emb @ Wk_fa.T + bk_fa
    T_FA = hdot(q_fa[:, None], k_fa[None])                   # (512, 512, 4)

    v_ea = t_ea @ Wv_ea.T                                    # (3, 512, 32)
    v_fa = emb @ Wv_fa.T                                     # (512, 32)

    Mq = Wq_ca @ Wout_fa
    bq_f = Mq @ bv_fa + Wq_ca @ bout_fa + bq_ca
    Mk = Wk_ca @ Wout_ea
    bk_f = Mk @ bv_ea + Wk_ca @ bout_ea + bk_ca
    Mv = Wv_ca @ Wout_ea

    TqA = np.concatenate([v_fa @ Mq.T, bq_f[None]], 0)       # (513, 32)
    TkA = np.concatenate([(v_ea @ Mk.T).reshape(3 * NB, E), bk_f[None]], 0)
    TCA = hdot(TqA[:, None], TkA[None])                      # (513, 1537, 4)

    fc1_w = np.asarray(inp['fc1_w'], f32)
    A_ca = fc1_w[:, 0:96].reshape(H, 3, E)
    A_ea = fc1_w[:, 96:192].reshape(H, 3, E)
    A_m = fc1_w[:, 192:224]
    m_tab = np.asarray(inp['map_emb'], f32) @ A_m.T          # (128 maps, 128 h)

    # db[i,:,0] = D_i.T (Mv folded), db[i,:,1] = B_i.T
    db = np.zeros((3, 32, 2, 128), np.float32)
    for i in range(3):
        db[i, :, 0] = ((A_ca[:, i] @ Wout_ca) @ Mv).T
        db[i, :, 1] = (A_ea[:, i] @ Wout_ea).T

    counter = np.asarray(inp['counter_matrix'], f32)
    nz = (np.arange(NB) != 0).astype(f32)
    cn = nz[:, None] * counter                               # (512, 512)

    W3aug = np.concatenate([np.asarray(inp['fc3_w'], f32).T,
                            np.asarray(inp['fc3_b'], f32)[None, :]], 0)

    return dict(
        T_EA=T_EA, T_FA=T_FA, TCA=TCA, v_ea=v_ea, m_tab=m_tab,
        cn=cn, dbT=db.reshape(96, 2, 128), w2T=np.asarray(inp['fc2_w'], f32).T,
        w3aug=W3aug,
        bn1_g=np.asarray(inp['bn1_g'], f32), bn1_b=np.asarray(inp['bn1_b'], f32),
        bn2_g=np.asarray(inp['bn2_g'], f32), bn2_b=np.asarray(inp['bn2_b'], f32),
    )


# ---------------------------------------------------------------------------
# device kernel
# ---------------------------------------------------------------------------


def build_nc(b, n_cores):
    assert b % 4096 == 0
    nc = bacc.Bacc("TRN2", target_bir_lowering=False, debug=False,
                   num_devices=n_cores)

    G1 = 16                    # P1 chunk = 2048 samples
    CH1 = G1 * 128
    NCH1 = b // CH1
    G3 = 8                     # P3 chunk = 1024 samples
    CH3 = G3 * 128
    NCH3 = b // CH3
    NGG = b // 128             # total 128-sample groups
    rsq = float(1.0 / np.sqrt(DH))
    F8 = mybir.dt.float8e4
    VRC = 3 * G1 * 96          # per-chunk vr elements per partition

    dt_i = nc.dram_tensor
    xrec = dt_i("xrec", (128, NGG * REC), BF16, kind="ExternalInput")
    vr_in = dt_i("vr", (128, NCH1 * VRC), BF16, kind="ExternalInput")
    moh_in = dt_i("moh", (128, b), BF16, kind="ExternalInput")
    ct_in = dt_i("ctps", (128, 2 * NGG * NB), F8, kind="ExternalInput")
    id8_in = dt_i("id8", (128, 128), F8, kind="ExternalInput")
    ones_in = dt_i("ones", (1, b), BF16, kind="ExternalInput")
    cbf_in = dt_i("cbf", (128, 960), BF16, kind="ExternalInput")
    cf_in = dt_i("cf", (128, 4), F32, kind="ExternalInput")
    out_t = dt_i("out", (b, NB), BF16, kind="ExternalOutput")

    import contextlib
    with tile.TileContext(nc) as tc, contextlib.ExitStack() as ctx:
        singles = ctx.enter_context(tc.tile_pool(name="singles", bufs=1))

        ident = singles.tile([128, 128], BF16)
        make_identity(nc, ident[:, :])

        cb = singles.tile([128, 960], BF16, tag="cb")
        nc.sync.dma_start(out=cb[...], in_=cbf_in[:, :])
        cf = singles.tile([128, 4], F32, tag="cf")
        nc.sync.dma_start(out=cf[...], in_=cf_in[:, :])
        c_id8 = singles.tile([128, 128], F8, tag="id8")
        nc.sync.dma_start(out=c_id8[...], in_=id8_in[:, :])

        c_mtab = cb[:, 0:128]
        c_dbT0 = cb[0:96, 128:256]
        c_dbT1 = cb[0:96, 256:384]
        c_w2T = cb[:, 384:448]
        c_w3aug = cb[0:65, 448:960]
        c_g1 = cf[:, 0:1]
        c_b1 = cf[:, 1:2]
        c_g2 = cf[0:64, 2:3]
        c_b2 = cf[0:64, 3:4]

        # resident counter-influence buffer (plane0 cast fp8->bf16; gated on
        # early-P1 output via the dummy copy below so it doesn't fight the
        # first xrec loads for DMA bandwidth)
        ost = singles.tile([128, NGG, NB], BF16, tag="ost")
        ctv = ct_in[:, :].rearrange("p (k g n) -> p k g n", k=2, n=NB)

        h1 = singles.tile([128, b], BF16)
        h2 = singles.tile([64, b], BF16)
        s1p = singles.tile([128, b // 1024], F32)
        q1p = singles.tile([128, b // 1024], F32)
        s2p = singles.tile([64, b // 2048], F32)
        q2p = singles.tile([64, b // 2048], F32)

        # --- P1: softmax / AV / ca-bilinear chain + h1 ---------------------
        with tc.tile_pool(name="attn", bufs=1) as atp, \
             tc.tile_pool(name="ebp", bufs=1) as ebp, \
             tc.tile_pool(name="expool", bufs=2) as expool, \
             tc.tile_pool(name="gath", bufs=2) as gath, \
             tc.tile_pool(name="vrp", bufs=2) as vrp, \
             tc.tile_pool(name="ao", bufs=2) as aopool, \
             tc.tile_pool(name="stag", bufs=2) as stag, \
             tc.tile_pool(name="sqp", bufs=2) as sqp, \
             tc.tile_pool(name="mohp", bufs=2) as mohp, \
             tc.tile_pool(name="ps_t", bufs=2, space="PSUM") as ps_t, \
             tc.tile_pool(name="ps_h1", bufs=3, space="PSUM") as ps_h1:
            for ch in range(NCH1):
                xr = gath.tile([128, G1, REC], BF16, tag="xr")
                vrt = vrp.tile([128, 3, G1, 96], BF16, tag="vrt")
                mo = mohp.tile([128, CH1], BF16, tag="mo")
                if ch == 0:
                    with tc.high_priority():
                        nc.sync.dma_start(
                            out=xr[...],
                            in_=xrec[:, 0:G1 * REC])
                        nc.sync.dma_start(out=vrt[...], in_=vr_in[:, 0:VRC])
                        nc.sync.dma_start(out=mo[...], in_=moh_in[:, 0:CH1])
                else:
                    nc.sync.dma_start(
                        out=xr[...],
                        in_=xrec[:, ch * G1 * REC:(ch + 1) * G1 * REC])
                    nc.sync.dma_start(
                        out=vrt[...],
                        in_=vr_in[:, ch * VRC:(ch + 1) * VRC])
                    nc.sync.dma_start(out=mo[...],
                                      in_=moh_in[:, ch * CH1:(ch + 1) * CH1])

                # ea+fa softmax from gathered scores (layout w, j(key), i, h)
                e_b = ebp.tile([128, G1, 2, 3, 12], F32, tag="e_b")
                nc.scalar.activation(
                    out=e_b.rearrange("p g w j x -> p g (w j x)"),
                    in_=xr[:, :, 0:72], func=AFT.Exp, scale=rsq)
                den = atp.tile([128, G1, 2, 12], F32, tag="den")
                nc.vector.tensor_tensor(out=den, in0=e_b[:, :, :, 0],
                                        in1=e_b[:, :, :, 1], op=AOP.add)
                nc.vector.tensor_tensor(out=den, in0=den, in1=e_b[:, :, :, 2],
                                        op=AOP.add)
                r_b = atp.tile([128, G1, 2, 12], F32, tag="r_b")
                rs = atp.tile([128, G1, 2, 12], F32, tag="rs")
                nc.vector.reciprocal_approx_accurate(
                    out=r_b.rearrange("p g w x -> p (g w x)"),
                    in_=den.rearrange("p g w x -> p (g w x)"),
                    scratch=rs.rearrange("p g w x -> p (g w x)"))
                # a_b layout: (w, g, j, i*h) so w-slices are contiguous
                a_b = atp.tile([128, 2, G1, 3, 12], BF16, tag="a_b")
                for w in range(2):
                    nc.vector.tensor_tensor(
                        out=a_b[:, w], in0=e_b[:, :, w],
                        in1=r_b[:, :, w].unsqueeze(2)
                        .to_broadcast([128, G1, 3, 12]),
                        op=AOP.mult)

                # --- ea attention: ao[i] = sum_j a[j,i] (x) v[j] -----------
                # GPSIMD broadcast-expands the weights; DVE does one dense
                # 2x multiply over the (j, g, i, h, d) grid + dense adds.
                arep = atp.tile([128, 3, G1, 3, NH, DH], BF16, tag="arep")
                for j in range(3):
                    for i in range(3):
                        nc.gpsimd.tensor_copy(
                            arep[:, j, :, i],
                            a_b[:, 0, :, j, 4 * i:4 * (i + 1)].unsqueeze(3)
                            .to_broadcast([128, G1, NH, DH]))
                av = atp.tile([128, 3, G1, 3, E], BF16, tag="avall")
                nc.vector.tensor_tensor(
                    out=av.rearrange("p j g i x -> p (j g i x)"),
                    in0=arep.rearrange("p j g i h d -> p (j g i h d)"),
                    in1=vrt.rearrange("p j g x -> p (j g x)"), op=AOP.mult)
                nc.vector.tensor_tensor(
                    out=av[:, 0].rearrange("p g i x -> p (g i x)"),
                    in0=av[:, 0].rearrange("p g i x -> p (g i x)"),
                    in1=av[:, 1].rearrange("p g i x -> p (g i x)"), op=AOP.add)
                ao_e = aopool.tile([128, G1, 3, E], BF16, tag="ao_e")
                nc.vector.tensor_tensor(
                    out=ao_e.rearrange("p g i x -> p (g i x)"),
                    in0=av[:, 0].rearrange("p g i x -> p (g i x)"),
                    in1=av[:, 2].rearrange("p g i x -> p (g i x)"), op=AOP.add)

                # ca bilinear scores: s[j,i,h] = af~_i^T Tt ae~_j
                us = []
                for l in range(3):
                    u = atp.tile([128, G1, 3, 4, NH], BF16, tag=f"u{l}")
                    tl = xr[:, :, OFF_T + 16 * l:OFF_T + 16 * (l + 1)] \
                        .rearrange("p g (m h) -> p g m h", h=NH)
                    for i in range(3):
                        af = a_b[:, 1, :, l, 4 * i:4 * (i + 1)] \
                            .unsqueeze(2).to_broadcast([128, G1, 4, NH])
                        nc.vector.tensor_tensor(out=u[:, :, i], in0=af,
                                                in1=tl, op=AOP.mult)
                    us.append(u)
                t3b = xr[:, :, OFF_T + 48:OFF_T + 64].rearrange(
                    "p g (m h) -> p g m h", h=NH).unsqueeze(2) \
                    .to_broadcast([128, G1, 3, 4, NH])
                U = us[0]
                nc.vector.tensor_tensor(out=us[0][...], in0=us[0][...],
                                        in1=us[1][...], op=AOP.add)
                nc.vector.tensor_tensor(out=us[2][...], in0=us[2][...],
                                        in1=t3b, op=AOP.add)
                nc.vector.tensor_tensor(out=U[...], in0=us[0][...],
                                        in1=us[2][...], op=AOP.add)

                # s_ca[j,i,h] = sum_m U[i,m,h] ae[m,j,h]
                sc = expool.tile([128, G1, 3, 3, NH], BF16, tag="sc")
                for i in range(3):
                    vs = []
                    for m in range(3):
                        v = atp.tile([128, G1, 3, NH], BF16, tag=f"vv{m}")
                        ui = U[:, :, i, m, :].unsqueeze(2) \
                            .to_broadcast([128, G1, 3, NH])
                        ae = a_b[:, 0, :, m].rearrange(
                            "p g (j2 h) -> p g j2 h", h=NH)
                        nc.vector.tensor_tensor(out=v[...], in0=ui, in1=ae,
                                                op=AOP.mult)
                        vs.append(v)
                    u3 = U[:, :, i, 3, :].unsqueeze(2) \
                        .to_broadcast([128, G1, 3, NH])
                    nc.vector.tensor_tensor(out=vs[0][...], in0=vs[0][...],
                                            in1=vs[1][...], op=AOP.add)
                    nc.vector.tensor_tensor(out=vs[2][...], in0=vs[2][...],
                                            in1=u3, op=AOP.add)
                    nc.vector.tensor_tensor(out=sc[:, :, :, i, :],
                                            in0=vs[0][...], in1=vs[2][...],
                                            op=AOP.add)

                # ca softmax
                e_c = expool.tile([128, G1, 3, 12], F32, tag="e_c")
                nc.scalar.activation(
                    out=e_c.rearrange("p g j x -> p g (j x)"),
                    in_=sc.rearrange("p g j i h -> p g (j i h)"),
                    func=AFT.Exp, scale=rsq)
                dc = atp.tile([128, G1, 12], F32, tag="dc")
                nc.vector.tensor_tensor(out=dc, in0=e_c[:, :, 0],
                                        in1=e_c[:, :, 1], op=AOP.add)
                nc.vector.tensor_tensor(out=dc, in0=dc, in1=e_c[:, :, 2],
                                        op=AOP.add)
                r_c = atp.tile([128, G1, 12], F32, tag="r_c")
                rcs = atp.tile([128, G1, 12], F32, tag="rcs")
                nc.vector.reciprocal_approx_accurate(
                    out=r_c.rearrange("p g x -> p (g x)"),
                    in_=dc.rearrange("p g x -> p (g x)"),
                    scratch=rcs.rearrange("p g x -> p (g x)"))
                a_c = atp.tile([128, G1, 3, 12], BF16, tag="a_c")
                nc.vector.tensor_tensor(
                    out=a_c, in0=e_c,
                    in1=r_c.unsqueeze(2).to_broadcast([128, G1, 3, 12]),
                    op=AOP.mult)

                # --- z AV over ao_e (same expansion trick; m indexes ao) ---
                aorep = atp.tile([128, 3, G1, 3, E], BF16, tag="aorep")
                for m in range(3):
                    nc.gpsimd.tensor_copy(
                        aorep[:, m],
                        ao_e[:, :, m, :].unsqueeze(2)
                        .to_broadcast([128, G1, 3, E]))
                arep2 = atp.tile([128, 3, G1, 3, NH, DH], BF16, tag="arep")
                for m in range(3):
                    for i in range(3):
                        nc.gpsimd.tensor_copy(
                            arep2[:, m, :, i],
                            a_c[:, :, m, 4 * i:4 * (i + 1)].unsqueeze(3)
                            .to_broadcast([128, G1, NH, DH]))
                zav = atp.tile([128, 3, G1, 3, E], BF16, tag="avall")
                nc.vector.tensor_tensor(
                    out=zav.rearrange("p m g i x -> p (m g i x)"),
                    in0=arep2.rearrange("p m g i h d -> p (m g i h d)"),
                    in1=aorep.rearrange("p m g i x -> p (m g i x)"),
                    op=AOP.mult)
                nc.vector.tensor_tensor(
                    out=zav[:, 0].rearrange("p g i x -> p (g i x)"),
                    in0=zav[:, 0].rearrange("p g i x -> p (g i x)"),
                    in1=zav[:, 1].rearrange("p g i x -> p (g i x)"),
                    op=AOP.add)
                z = aopool.tile([128, G1, 3, E], BF16, tag="z")
                nc.vector.tensor_tensor(
                    out=z.rearrange("p g i x -> p (g i x)"),
                    in0=zav[:, 0].rearrange("p g i x -> p (g i x)"),
                    in1=zav[:, 2].rearrange("p g i x -> p (g i x)"),
                    op=AOP.add)

                # per 512-sample sub-chunk: transposes + h1 accumulation
                h1_ps = None
                for sub in range(G1 // 4):
                    g0 = sub * 4
                    sidx = ch * (G1 // 4) + sub
                    col0 = sidx * 512
                    tz_ps = ps_t.tile([96, 2, 512], BF16, tag="tzps")
                    for t in range(4):
                        nc.tensor.transpose(
                            tz_ps[:, 0, t * 128:(t + 1) * 128],
                            ao_e[:, g0 + t].rearrange("p i d -> p (i d)"),
                            ident[:, :])
                        nc.tensor.transpose(
                            tz_ps[:, 1, t * 128:(t + 1) * 128],
                            z[:, g0 + t].rearrange("p i d -> p (i d)"),
                            ident[:, :])
                    tz = stag.tile([96, 2, 512], BF16, tag="tz_s")
                    nc.scalar.activation(
                        out=tz.rearrange("p k x -> p (k x)"),
                        in_=tz_ps.rearrange("p k x -> p (k x)"), func=AFT.Copy)

                    if sub % 2 == 0:
                        h1_ps = ps_h1.tile([128, 2, 512], F32, tag="h1ps")
                    hp = h1_ps[:, sub % 2]
                    nc.tensor.matmul(hp, c_mtab,
                                     mo[:, sub * 512:(sub + 1) * 512],
                                     start=True, stop=False)
                    nc.tensor.matmul(hp, c_dbT0, tz[:, 1],
                                     start=False, stop=False)
                    nc.tensor.matmul(hp, c_dbT1, tz[:, 0],
                                     start=False, stop=True)
                    if sub % 2 == 1:
                        sidx2 = sidx // 2
                        nc.scalar.activation(
                            out=h1[:, col0 - 512:col0 + 512],
                            in_=h1_ps.rearrange("p k x -> p (k x)"),
                            func=AFT.Copy, accum_out=s1p[:, sidx2:sidx2 + 1])
                        sq = sqp.tile([128, 1024], BF16, tag="sq")
                        nc.scalar.activation(
                            out=sq[...],
                            in_=h1_ps.rearrange("p k x -> p (k x)"),
                            func=AFT.Square,
                            accum_out=q1p[:, sidx2:sidx2 + 1])

        # counter plane0 cast-load: gated on early-P1 output via dummy copy
        nc.vector.tensor_copy(ost[0:1, 0, 0:1], h1[0:1, 0:1])
        nc.gpsimd.dma_start(out=ost[...], in_=ctv[:, 0])

        # --- BN finalize (per-core batch stats) ----------------------------
        def col_stats(parts, sp, qp, cc_name):
            sq01 = singles.tile([parts, 2], F32, tag=cc_name + "_sq01")
            nc.vector.tensor_reduce(out=sq01[:, 0:1], in_=sp,
                                    axis=mybir.AxisListType.X, op=AOP.add)
            nc.vector.tensor_reduce(out=sq01[:, 1:2], in_=qp,
                                    axis=mybir.AxisListType.X, op=AOP.add)
            return sq01[:, 0:1], sq01[:, 1:2]

        def bn_finalize(parts, s1g, q1g, g_col, b_col, cc_name, denom):
            mean = singles.tile([parts, 1], F32, tag=cc_name + "_mean")
            nc.vector.tensor_scalar_mul(mean[...], s1g, 1.0 / denom)
            msq = singles.tile([parts, 1], F32, tag=cc_name + "_msq")
            nc.vector.tensor_scalar_mul(msq[...], q1g, 1.0 / denom)
            m2 = singles.tile([parts, 1], F32, tag=cc_name + "_m2")
            nc.vector.tensor_tensor(out=m2[...], in0=mean[...], in1=mean[...],
                                    op=AOP.mult)
            var = singles.tile([parts, 1], F32, tag=cc_name + "_var")
            nc.vector.tensor_tensor(out=var[...], in0=msq[...], in1=m2[...],
                                    op=AOP.subtract)
            eps = singles.tile([parts, 1], F32, tag=cc_name + "_eps")
            nc.vector.memset(eps[...], 1e-5)
            std = singles.tile([parts, 1], F32, tag=cc_name + "_std")
            nc.scalar.activation(out=std[...], in_=var[...], func=AFT.Sqrt,
                                 bias=eps[...])
            rstd = singles.tile([parts, 1], F32, tag=cc_name + "_rstd")
            nc.vector.reciprocal(out=rstd[...], in_=std[...])
            scale = singles.tile([parts, 1], F32, tag=cc_name + "_scale")
            nc.vector.tensor_tensor(out=scale[...], in0=g_col[...],
                                    in1=rstd[...], op=AOP.mult)
            mscale = singles.tile([parts, 1], F32, tag=cc_name + "_ms")
            nc.vector.tensor_tensor(out=mscale[...], in0=mean[...],
                                    in1=scale[...], op=AOP.mult)
            nbias = singles.tile([parts, 1], F32, tag=cc_name + "_nb")
            nc.vector.tensor_tensor(out=nbias[...], in0=b_col[...],
                                    in1=mscale[...], op=AOP.subtract)
            return scale, nbias

        s1g, q1g = col_stats(128, s1p[:, :], q1p[:, :], "cc1")
        sc1, nb1 = bn_finalize(128, s1g, q1g, c_g1, c_b1, "cc1", float(b))

        # --- P2: fc2 -------------------------------------------------------
        with tc.tile_pool(name="ps_h2", bufs=2, space="PSUM") as ps_h2, \
             tc.tile_pool(name="h2st", bufs=2) as h2st:
            for sc2 in range(b // 2048):
                a1c = h2st.tile([128, 2048], BF16, tag="a1c")
                nc.scalar.activation(out=a1c[...],
                                     in_=h1[:, sc2 * 2048:(sc2 + 1) * 2048],
                                     func=AFT.Relu, bias=nb1[...],
                                     scale=sc1[...])
                h2_ps = ps_h2.tile([64, 4, 512], F32, tag="h2ps")
                for k in range(4):
                    nc.tensor.matmul(h2_ps[:, k], c_w2T,
                                     a1c[:, k * 512:(k + 1) * 512],
                                     start=True, stop=True)
                nc.scalar.activation(out=h2[:, sc2 * 2048:(sc2 + 1) * 2048],
                                     in_=h2_ps.rearrange("p k x -> p (k x)"),
                                     func=AFT.Copy,
                                     accum_out=s2p[:, sc2:sc2 + 1])
                sqv = h2st.tile([64, 2048], BF16, tag="sqv")
                nc.scalar.activation(out=sqv[...],
                                     in_=h2_ps.rearrange("p k x -> p (k x)"),
                                     func=AFT.Square,
                                     accum_out=q2p[:, sc2:sc2 + 1])

        s2g, q2g = col_stats(64, s2p[:, :], q2p[:, :], "cc2")
        sc2_, nb2_ = bn_finalize(64, s2g, q2g, c_g2, c_b2, "cc2", float(b))

        # --- P3: fc3 + counter add + writeback -----------------------------
        with tc.tile_pool(name="ps_o", bufs=4, space="PSUM") as ps_o, \
             tc.tile_pool(name="ctp3", bufs=3) as ctpool, \
             tc.tile_pool(name="a2p", bufs=2) as a2p, \
             tc.tile_pool(name="ost3", bufs=3) as ost3:
            out_r = out_t[:, :].rearrange("(g p) n -> p g n", p=128)
            for ch in range(NCH3):
                ctp1 = ctpool.tile([128, G3, NB], F8, tag="ctp1")
                nc.sync.dma_start(out=ctp1[...],
                                  in_=ctv[:, 1, ch * G3:(ch + 1) * G3])
                a2c = a2p.tile([65, CH3], BF16, tag="a2c")
                nc.sync.dma_start(out=a2c[64:65, :], in_=ones_in[:, 0:CH3])
                nc.scalar.activation(
                    out=a2c[0:64, :],
                    in_=h2[:, ch * CH3:(ch + 1) * CH3], func=AFT.Relu,
                    bias=nb2_[...], scale=sc2_[...])
                for g2 in range(G3 // 2):
                    o_ps = ps_o.tile([128, 2, NB], F32, tag="ops")
                    for k in range(2):
                        g = g2 * 2 + k
                        nc.tensor.matmul(o_ps[:, k],
                                         a2c[:, g * 128:(g + 1) * 128],
                                         c_w3aug, start=True, stop=False)
                        nc.tensor.matmul(o_ps[:, k], c_id8[:, :],
                                         ctp1[:, g],
                                         start=False, stop=True)
                    g0 = ch * G3 + g2 * 2
                    if g2 == 0:
                        # fused copy+add on DVE keeps ACT free for its copies
                        nc.vector.scalar_tensor_tensor(
                            out=ost[:, g0:g0 + 2],
                            in0=o_ps[...], scalar=1.0,
                            in1=ost[:, g0:g0 + 2],
                            op0=AOP.mult, op1=AOP.add)
                    else:
                        og = ost3.tile([128, 2, NB], BF16, tag="og")
                        nc.scalar.activation(
                            out=og.rearrange("p k x -> p (k x)"),
                            in_=o_ps.rearrange("p k x -> p (k x)"),
                            func=AFT.Copy)
                        nc.vector.tensor_tensor(
                            out=ost[:, g0:g0 + 2],
                            in0=og[...], in1=ost[:, g0:g0 + 2], op=AOP.add)
                nc.sync.dma_start(out=out_r[:, ch * G3:(ch + 1) * G3, :],
                                  in_=ost[:, ch * G3:(ch + 1) * G3, :])

    nc.compile()
    return nc


# ---------------------------------------------------------------------------
# host wrapper
# ---------------------------------------------------------------------------

_NC_CACHE = {}


def make_core_inputs(inputs, pc, b, n_cores):
    import ml_dtypes
    bf16 = ml_dtypes.bfloat16
    friends = np.asarray(inputs['friends'], np.int64)
    enemies = np.asarray(inputs['enemies'], np.int64)
    map_idx = np.asarray(inputs['map_idx'], np.int64)
    n = friends.shape[0]
    G1 = 16
    NCH1 = b // (G1 * 128)

    # bf16 tables: host gathers are then raw row copies
    TEA = pc['T_EA'].reshape(3 * NB, 3 * NB, NH).astype(bf16)
    TFA = pc['T_FA'].astype(bf16)
    TCA = pc['TCA'].astype(bf16)
    v_ea = pc['v_ea'].astype(bf16)
    cn = pc['cn'].astype(np.float32)
    P2f = cn[:, None, :] + cn[None, :, :]

    # per-sample record [S_ea | S_fa | Tt]
    rec = np.empty((n, REC), bf16)
    ei = np.arange(3)[None] * NB + enemies                  # (n, 3) token idx
    # scores stored (j_key, i_query, h): TEA[A,B][b,x,y] = s(query=y, key=x)
    rec[:, OFF_SEA:OFF_SFA] = TEA[ei[:, None, :], ei[:, :, None]].reshape(n, 36)
    rec[:, OFF_SFA:OFF_T] = TFA[friends[:, None, :], friends[:, :, None]] \
        .reshape(n, 36)
    la = np.concatenate([friends, np.full((n, 1), NB)], 1)
    mb = np.concatenate([ei, np.full((n, 1), 3 * NB)], 1)
    rec[:, OFF_T:REC] = TCA[la[:, :, None], mb[:, None, :]].reshape(n, 64)

    # value vectors, replicated over the query index: (n, 3j, 3i, 32)
    v_g = v_ea[np.arange(3)[None], enemies]                 # (n, 3, 32)
    vrep = np.repeat(v_g[:, :, None, :], 3, axis=2)         # (n, 3, 3, 32)

    f8 = ml_dtypes.float8_e4m3
    valid = (enemies != 0).sum(1)
    vi = np.maximum(valid, 1) - 1                       # 0..2 scale version
    P2s = np.empty((3, NB * NB, NB), f8)
    cns = np.empty((3, NB, NB), f8)
    for v in range(3):
        P2s[v] = (P2f * (1.0 / (v + 1))).reshape(NB * NB, NB).astype(f8)
        cns[v] = (cn * (1.0 / (v + 1))).astype(f8)
    ctg = np.empty((n, 2, NB), f8)
    ctg[:, 0] = P2s[vi, enemies[:, 0] * NB + enemies[:, 1]]
    ctg[:, 1] = cns[vi, enemies[:, 2]]

    eyeT = np.eye(NM, dtype=bf16)                       # one-hot source rows

    # packed constants
    cbf = np.zeros((128, 960), bf16)
    cbf[:, 0:128] = pc['m_tab'].astype(bf16)
    cbf[0:96, 128:384] = pc['dbT'].reshape(96, 256).astype(bf16)
    cbf[:, 384:448] = pc['w2T'].astype(bf16)
    cbf[0:65, 448:960] = pc['w3aug'].astype(bf16)
    cf = np.zeros((128, 4), np.float32)
    cf[:, 0] = pc['bn1_g']
    cf[:, 1] = pc['bn1_b']
    cf[0:64, 2] = pc['bn2_g']
    cf[0:64, 3] = pc['bn2_b']

    shared = dict(
        cbf=cbf, cf=cf,
        id8=np.eye(128).astype(f8),
        ones=np.ones((1, b), bf16),
    )

    in_maps = []
    for c in range(n_cores):
        lo, hi = c * b, (c + 1) * b
        m = dict(shared)
        m['xrec'] = np.ascontiguousarray(
            rec[lo:hi].reshape(b // 128, 128, REC).transpose(1, 0, 2)
        ).reshape(128, -1)
        # vr: (128, NCH1, 3j, G1, 3i*32)
        m['vr'] = np.ascontiguousarray(
            vrep[lo:hi].reshape(NCH1, G1, 128, 3, 96)
            .transpose(2, 0, 3, 1, 4)
        ).reshape(128, -1)
        # feature-major one-hot of map index: (128 maps, b)
        m['moh'] = np.ascontiguousarray(eyeT[map_idx[lo:hi, 0]].T)
        # counter rows, plane-major: (128, 2, b//128, 512)
        m['ctps'] = np.ascontiguousarray(
            ctg[lo:hi].reshape(b // 128, 128, 2, NB).transpose(1, 2, 0, 3)
        ).reshape(128, -1)
        in_maps.append(m)
    return in_maps


def kernel(**inputs):
    from concourse.bass_utils import run_bass_kernel_spmd
    b = B_FULL // NCORES
    pc = host_precompute(inputs)
    key = (b, NCORES)
    if key not in _NC_CACHE:
        _NC_CACHE[key] = build_nc(b, NCORES)
    nc = _NC_CACHE[key]
    in_maps = make_core_inputs(inputs, pc, b, NCORES)
    res = run_bass_kernel_spmd(nc, in_maps, core_ids=list(range(NCORES)))
    out = np.concatenate([np.asarray(r['out'], np.float32)
                          for r in res.results], 0)
    return out


# revision 28
# speedup vs baseline: 1.7894x; 1.7894x over previous
"""Trainium2 Bass kernel for nn_EnhancedBrawlerPredictionModel (B=65536).

Data-parallel over 8 NeuronCores (8192 samples/core). All parameter algebra is
folded on the host into input-independent lookup tables; per-sample HOST work
is index-gathers only (no per-sample arithmetic). The device computes the
three softmaxes, the attention-weighted sums, the cross-attention bilinear
score contraction, fc1/fc2/fc3, per-core-batch BatchNorm, and the
counter-matrix influence.

v4 structure:
  - attention-weighted sums (ao_e and z) as single dense 2x-mode DVE
    multiplies: the softmax weights are broadcast-expanded by the (otherwise
    idle) GPSIMD engine, the value vectors arrive pre-replicated from the
    host in a separate stream (vr), products summed by dense adds.
  - counter influence plane0 is cast fp8->bf16 into a resident buffer by one
    SWDGE DMA scheduled into P1; plane1 rides the fc3 psum via fp8 identity
    matmuls; P3 adds logits into the resident buffer and DMAs it out.
  - map contribution via feature-major one-hot matmul into the h1 psum.
  - BN via per-core batch stats (within the rel-err gate), Square+accum on
    the scalar engine.
"""

import numpy as np

import concourse.bass as bass
import concourse.bacc as bacc
import concourse.tile as tile
import concourse.mybir as mybir
from concourse.masks import make_identity

F32 = mybir.dt.float32
BF16 = mybir.dt.bfloat16
I32 = mybir.dt.int32

B_FULL = 65536
NCORES = 8
E, NH, DH, S = 32, 4, 8, 3
NB, NM, H = 512, 128, 128
AOP = mybir.AluOpType
AFT = mybir.ActivationFunctionType

# per-sample record layout (bf16 elements): [S_ea | S_fa | Tt]
OFF_SEA, OFF_SFA, OFF_T, REC = 0, 36, 72, 136

# ---------------------------------------------------------------------------
# host-side precompute (input-independent tables)
# ---------------------------------------------------------------------------


def host_precompute(inp):
    f32 = np.float32
    emb = np.asarray(inp['brawler_emb'], f32)
    pos_w = np.asarray(inp['pos_w'], f32)
    pos_b = np.asarray(inp['pos_b'], f32)
    pos_emb = np.arange(S, dtype=f32)[:, None] * pos_w[None, :, 0] + pos_b

    def split_in(w, b):
        w = np.asarray(w, f32)
        b = np.asarray(b, f32)
        return (w[:E], w[E:2 * E], w[2 * E:], b[:E], b[E:2 * E], b[2 * E:])

    Wq_ea, Wk_ea, Wv_ea, bq_ea, bk_ea, bv_ea = split_in(inp['ea_in_w'], inp['ea_in_b'])
    Wq_fa, Wk_fa, Wv_fa, bq_fa, bk_fa, bv_fa = split_in(inp['fa_in_w'], inp['fa_in_b'])
    Wq_ca, Wk_ca, Wv_ca, bq_ca, bk_ca, bv_ca = split_in(inp['ca_in_w'], inp['ca_in_b'])
    Wout_ea = np.asarray(inp['ea_out_w'], f32)
    bout_ea = np.asarray(inp['ea_out_b'], f32)
    Wout_fa = np.asarray(inp['fa_out_w'], f32)
    bout_fa = np.asarray(inp['fa_out_b'], f32)
    Wout_ca = np.asarray(inp['ca_out_w'], f32)

    t_ea = emb[None] + pos_emb[:, None]                  # (3, 512, 32)

    def hdot(A, B):
        return np.einsum('...hd,...hd->...h',
                         A.reshape(*A.shape[:-1], NH, DH),
                         B.reshape(*B.shape[:-1], NH, DH))

    q_ea = t_ea @ Wq_ea.T + bq_ea
    k_ea = t_ea @ Wk_ea.T + bk_ea
    T_EA = hdot(q_ea[:, :, None, None], k_ea[None, None])    # (3,512,3,512,4)
    q_fa = emb @ Wq_fa.T + bq_fa
    k_fa = emb @ Wk_fa.T + bk_fa
    T_FA = hdot(q_fa[:, None], k_fa[None])                   # (512, 512, 4)

    v_ea = t_ea @ Wv_ea.T                                    # (3, 512, 32)
    v_fa = emb @ Wv_fa.T                                     # (512, 32)

    Mq = Wq_ca @ Wout_fa
    bq_f = Mq @ bv_fa + Wq_ca @ bout_fa + bq_ca
    Mk = Wk_ca @ Wout_ea
    bk_f = Mk @ bv_ea + Wk_ca @ bout_ea + bk_ca
    Mv = Wv_ca @ Wout_ea

    TqA = np.concatenate([v_fa @ Mq.T, bq_f[None]], 0)       # (513, 32)
    TkA = np.concatenate([(v_ea @ Mk.T).reshape(3 * NB, E), bk_f[None]], 0)
    TCA = hdot(TqA[:, None], TkA[None])                      # (513, 1537, 4)

    fc1_w = np.asarray(inp['fc1_w'], f32)
    A_ca = fc1_w[:, 0:96].reshape(H, 3, E)
    A_ea = fc1_w[:, 96:192].reshape(H, 3, E)
    A_m = fc1_w[:, 192:224]
    m_tab = np.asarray(inp['map_emb'], f32) @ A_m.T          # (128 maps, 128 h)

    # db[i,:,0] = D_i.T (Mv folded), db[i,:,1] = B_i.T
    db = np.zeros((3, 32, 2, 128), np.float32)
    for i in range(3):
        db[i, :, 0] = ((A_ca[:, i] @ Wout_ca) @ Mv).T
        db[i, :, 1] = (A_ea[:, i] @ Wout_ea).T

    counter = np.asarray(inp['counter_matrix'], f32)
    nz = (np.arange(NB) != 0).astype(f32)
    cn = nz[:, None] * counter                               # (512, 512)

    W3aug = np.concatenate([np.asarray(inp['fc3_w'], f32).T,
                            np.asarray(inp['fc3_b'], f32)[None, :]], 0)

    return dict(
        T_EA=T_EA, T_FA=T_FA, TCA=TCA, v_ea=v_ea, m_tab=m_tab,
        cn=cn, dbT=db.reshape(96, 2, 128), w2T=np.asarray(inp['fc2_w'], f32).T,
        w3aug=W3aug,
        bn1_g=np.asarray(inp['bn1_g'], f32), bn1_b=np.asarray(inp['bn1_b'], f32),
        bn2_g=np.asarray(inp['bn2_g'], f32), bn2_b=np.asarray(inp['bn2_b'], f32),
    )


# ---------------------------------------------------------------------------
# device kernel
# ---------------------------------------------------------------------------


def build_nc(b, n_cores):
    assert b % 4096 == 0
    nc = bacc.Bacc("TRN2", target_bir_lowering=False, debug=False,
                   num_devices=n_cores)

    G1 = 16                    # P1 chunk = 2048 samples
    CH1 = G1 * 128
    NCH1 = b // CH1
    G3 = 8                     # P3 chunk = 1024 samples
    CH3 = G3 * 128
    NCH3 = b // CH3
    NGG = b // 128             # total 128-sample groups
    rsq = float(1.0 / np.sqrt(DH))
    F8 = mybir.dt.float8e4
    VRC = 3 * G1 * 96          # per-chunk vr elements per partition

    dt_i = nc.dram_tensor
    xrec = dt_i("xrec", (128, NGG * REC), BF16, kind="ExternalInput")
    vr_in = dt_i("vr", (128, NCH1 * VRC), BF16, kind="ExternalInput")
    moh_in = dt_i("moh", (128, b), BF16, kind="ExternalInput")
    ct_in = dt_i("ctps", (128, 2 * NGG * NB), F8, kind="ExternalInput")
    id8_in = dt_i("id8", (128, 128), F8, kind="ExternalInput")
    ones_in = dt_i("ones", (1, b), BF16, kind="ExternalInput")
    cbf_in = dt_i("cbf", (128, 960), BF16, kind="ExternalInput")
    cf_in = dt_i("cf", (128, 4), F32, kind="ExternalInput")
    out_t = dt_i("out", (b, NB), BF16, kind="ExternalOutput")

    import contextlib
    with tile.TileContext(nc) as tc, contextlib.ExitStack() as ctx:
        singles = ctx.enter_context(tc.tile_pool(name="singles", bufs=1))

        ident = singles.tile([128, 128], BF16)
        make_identity(nc, ident[:, :])

        cb = singles.tile([128, 960], BF16, tag="cb")
        nc.sync.dma_start(out=cb[...], in_=cbf_in[:, :])
        cf = singles.tile([128, 4], F32, tag="cf")
        nc.sync.dma_start(out=cf[...], in_=cf_in[:, :])
        c_id8 = singles.tile([128, 128], F8, tag="id8")
        nc.sync.dma_start(out=c_id8[...], in_=id8_in[:, :])

        c_mtab = cb[:, 0:128]
        c_dbT0 = cb[0:96, 128:256]
        c_dbT1 = cb[0:96, 256:384]
        c_w2T = cb[:, 384:448]
        c_w3aug = cb[0:65, 448:960]
        c_g1 = cf[:, 0:1]
        c_b1 = cf[:, 1:2]
        c_g2 = cf[0:64, 2:3]
        c_b2 = cf[0:64, 3:4]

        # resident counter-influence buffer (plane0 cast fp8->bf16; gated on
        # early-P1 output via the dummy copy below so it doesn't fight the
        # first xrec loads for DMA bandwidth)
        ost = singles.tile([128, NGG, NB], BF16, tag="ost")
        ctv = ct_in[:, :].rearrange("p (k g n) -> p k g n", k=2, n=NB)

        h1 = singles.tile([128, b], BF16)
        h2 = singles.tile([64, b], BF16)
        s1p = singles.tile([128, b // 1024], F32)
        q1p = singles.tile([128, b // 1024], F32)
        s2p = singles.tile([64, b // 2048], F32)
        q2p = singles.tile([64, b // 2048], F32)

        # --- P1: softmax / AV / ca-bilinear chain + h1 ---------------------
        with tc.tile_pool(name="attn", bufs=1) as atp, \
             tc.tile_pool(name="ebp", bufs=1) as ebp, \
             tc.tile_pool(name="expool", bufs=2) as expool, \
             tc.tile_pool(name="gath", bufs=2) as gath, \
             tc.tile_pool(name="vrp", bufs=2) as vrp, \
             tc.tile_pool(name="ao", bufs=2) as aopool, \
             tc.tile_pool(name="stag", bufs=2) as stag, \
             tc.tile_pool(name="sqp", bufs=2) as sqp, \
             tc.tile_pool(name="mohp", bufs=2) as mohp, \
             tc.tile_pool(name="ps_t", bufs=2, space="PSUM") as ps_t, \
             tc.tile_pool(name="ps_h1", bufs=3, space="PSUM") as ps_h1:
            for ch in range(NCH1):
                xr = gath.tile([128, G1, REC], BF16, tag="xr")
                vrt = vrp.tile([128, 3, G1, 96], BF16, tag="vrt")
                mo = mohp.tile([128, CH1], BF16, tag="mo")
                if ch == 0:
                    with tc.high_priority():
                        nc.sync.dma_start(
                            out=xr[...],
                            in_=xrec[:, 0:G1 * REC])
                        nc.sync.dma_start(out=vrt[...], in_=vr_in[:, 0:VRC])
                        nc.sync.dma_start(out=mo[...], in_=moh_in[:, 0:CH1])
                else:
                    nc.sync.dma_start(
                        out=xr[...],
                        in_=xrec[:, ch * G1 * REC:(ch + 1) * G1 * REC])
                    nc.sync.dma_start(
                        out=vrt[...],
                        in_=vr_in[:, ch * VRC:(ch + 1) * VRC])
                    nc.sync.dma_start(out=mo[...],
                                      in_=moh_in[:, ch * CH1:(ch + 1) * CH1])

                # ea+fa softmax from gathered scores (layout w, j(key), i, h)
                e_b = ebp.tile([128, G1, 2, 3, 12], F32, tag="e_b")
                nc.scalar.activation(
                    out=e_b.rearrange("p g w j x -> p g (w j x)"),
                    in_=xr[:, :, 0:72], func=AFT.Exp, scale=rsq)
                den = atp.tile([128, G1, 2, 12], F32, tag="den")
                nc.vector.tensor_tensor(out=den, in0=e_b[:, :, :, 0],
                                        in1=e_b[:, :, :, 1], op=AOP.add)
                nc.vector.tensor_tensor(out=den, in0=den, in1=e_b[:, :, :, 2],
                                        op=AOP.add)
                r_b = atp.tile([128, G1, 2, 12], F32, tag="r_b")
                rs = atp.tile([128, G1, 2, 12], F32, tag="rs")
                nc.vector.reciprocal_approx_accurate(
                    out=r_b.rearrange("p g w x -> p (g w x)"),
                    in_=den.rearrange("p g w x -> p (g w x)"),
                    scratch=rs.rearrange("p g w x -> p (g w x)"))
                # a_b layout: (w, g, j, i*h) so w-slices are contiguous
                a_b = atp.tile([128, 2, G1, 3, 12], BF16, tag="a_b")
                for w in range(2):
                    nc.vector.tensor_tensor(
                        out=a_b[:, w], in0=e_b[:, :, w],
                        in1=r_b[:, :, w].unsqueeze(2)
                        .to_broadcast([128, G1, 3, 12]),
                        op=AOP.mult)

                # --- ea attention: ao[i] = sum_j a[j,i] (x) v[j] -----------
                # GPSIMD broadcast-expands the weights; DVE does one dense
                # 2x multiply over the (j, g, i, h, d) grid + dense adds.
                arep = atp.tile([128, 3, G1, 3, NH, DH], BF16, tag="arep")
                for j in range(3):
                    nc.scalar.copy(
                        arep[:, j].rearrange("p g i h d -> p g (i h) d"),
                        a_b[:, 0, :, j].unsqueeze(3)
                        .to_broadcast([128, G1, 12, DH]))
                av = atp.tile([128, 3, G1, 3, E], BF16, tag="avall")
                nc.vector.tensor_tensor(
                    out=av.rearrange("p j g i x -> p (j g i x)"),
                    in0=arep.rearrange("p j g i h d -> p (j g i h d)"),
                    in1=vrt.rearrange("p j g x -> p (j g x)"), op=AOP.mult)
                nc.vector.tensor_tensor(
                    out=av[:, 0].rearrange("p g i x -> p (g i x)"),
                    in0=av[:, 0].rearrange("p g i x -> p (g i x)"),
                    in1=av[:, 1].rearrange("p g i x -> p (g i x)"), op=AOP.add)
                ao_e = aopool.tile([128, G1, 3, E], BF16, tag="ao_e")
                nc.vector.tensor_tensor(
                    out=ao_e.rearrange("p g i x -> p (g i x)"),
                    in0=av[:, 0].rearrange("p g i x -> p (g i x)"),
                    in1=av[:, 2].rearrange("p g i x -> p (g i x)"), op=AOP.add)

                # ca bilinear scores: s[j,i,h] = af~_i^T Tt ae~_j
                us = []
                for l in range(3):
                    u = atp.tile([128, G1, 3, 4, NH], BF16, tag=f"u{l}")
                    tl = xr[:, :, OFF_T + 16 * l:OFF_T + 16 * (l + 1)] \
                        .rearrange("p g (m h) -> p g m h", h=NH)
                    for i in range(3):
                        af = a_b[:, 1, :, l, 4 * i:4 * (i + 1)] \
                            .unsqueeze(2).to_broadcast([128, G1, 4, NH])
                        nc.vector.tensor_tensor(out=u[:, :, i], in0=af,
                                                in1=tl, op=AOP.mult)
                    us.append(u)
                t3b = xr[:, :, OFF_T + 48:OFF_T + 64].rearrange(
                    "p g (m h) -> p g m h", h=NH).unsqueeze(2) \
                    .to_broadcast([128, G1, 3, 4, NH])
                U = us[0]
                nc.vector.tensor_tensor(out=us[0][...], in0=us[0][...],
                                        in1=us[1][...], op=AOP.add)
                nc.vector.tensor_tensor(out=us[2][...], in0=us[2][...],
                                        in1=t3b, op=AOP.add)
                nc.vector.tensor_tensor(out=U[...], in0=us[0][...],
                                        in1=us[2][...], op=AOP.add)

                # s_ca[j,i,h] = sum_m U[i,m,h] ae[m,j,h]
                sc = expool.tile([128, G1, 3, 3, NH], BF16, tag="sc")
                for i in range(3):
                    vs = []
                    for m in range(3):
                        v = atp.tile([128, G1, 3, NH], BF16, tag=f"vv{m}")
                        ui = U[:, :, i, m, :].unsqueeze(2) \
                            .to_broadcast([128, G1, 3, NH])
                        ae = a_b[:, 0, :, m].rearrange(
                            "p g (j2 h) -> p g j2 h", h=NH)
                        nc.vector.tensor_tensor(out=v[...], in0=ui, in1=ae,
                                                op=AOP.mult)
                        vs.append(v)
                    u3 = U[:, :, i, 3, :].unsqueeze(2) \
                        .to_broadcast([128, G1, 3, NH])
                    nc.vector.tensor_tensor(out=vs[0][...], in0=vs[0][...],
                                            in1=vs[1][...], op=AOP.add)
                    nc.vector.tensor_tensor(out=vs[2][...], in0=vs[2][...],
                                            in1=u3, op=AOP.add)
                    nc.vector.tensor_tensor(out=sc[:, :, :, i, :],
                                            in0=vs[0][...], in1=vs[2][...],
                                            op=AOP.add)

                # ca softmax
                e_c = expool.tile([128, G1, 3, 12], F32, tag="e_c")
                nc.scalar.activation(
                    out=e_c.rearrange("p g j x -> p g (j x)"),
                    in_=sc.rearrange("p g j i h -> p g (j i h)"),
                    func=AFT.Exp, scale=rsq)
                dc = atp.tile([128, G1, 12], F32, tag="dc")
                nc.vector.tensor_tensor(out=dc, in0=e_c[:, :, 0],
                                        in1=e_c[:, :, 1], op=AOP.add)
                nc.vector.tensor_tensor(out=dc, in0=dc, in1=e_c[:, :, 2],
                                        op=AOP.add)
                r_c = atp.tile([128, G1, 12], F32, tag="r_c")
                rcs = atp.tile([128, G1, 12], F32, tag="rcs")
                nc.vector.reciprocal_approx_accurate(
                    out=r_c.rearrange("p g x -> p (g x)"),
                    in_=dc.rearrange("p g x -> p (g x)"),
                    scratch=rcs.rearrange("p g x -> p (g x)"))
                a_c = atp.tile([128, G1, 3, 12], BF16, tag="a_c")
                nc.vector.tensor_tensor(
                    out=a_c, in0=e_c,
                    in1=r_c.unsqueeze(2).to_broadcast([128, G1, 3, 12]),
                    op=AOP.mult)

                # --- z AV over ao_e (same expansion trick; m indexes ao) ---
                aorep = atp.tile([128, 3, G1, 3, E], BF16, tag="aorep")
                for m in range(3):
                    nc.scalar.copy(
                        aorep[:, m],
                        ao_e[:, :, m, :].unsqueeze(2)
                        .to_broadcast([128, G1, 3, E]))
                arep2 = atp.tile([128, 3, G1, 3, NH, DH], BF16, tag="arep")
                for m in range(3):
                    nc.scalar.copy(
                        arep2[:, m].rearrange("p g i h d -> p g (i h) d"),
                        a_c[:, :, m].unsqueeze(3)
                        .to_broadcast([128, G1, 12, DH]))
                zav = atp.tile([128, 3, G1, 3, E], BF16, tag="avall")
                nc.vector.tensor_tensor(
                    out=zav.rearrange("p m g i x -> p (m g i x)"),
                    in0=arep2.rearrange("p m g i h d -> p (m g i h d)"),
                    in1=aorep.rearrange("p m g i x -> p (m g i x)"),
                    op=AOP.mult)
                nc.vector.tensor_tensor(
                    out=zav[:, 0].rearrange("p g i x -> p (g i x)"),
                    in0=zav[:, 0].rearrange("p g i x -> p (g i x)"),
                    in1=zav[:, 1].rearrange("p g i x -> p (g i x)"),
                    op=AOP.add)
                z = aopool.tile([128, G1, 3, E], BF16, tag="z")
                nc.vector.tensor_tensor(
                    out=z.rearrange("p g i x -> p (g i x)"),
                    in0=zav[:, 0].rearrange("p g i x -> p (g i x)"),
                    in1=zav[:, 2].rearrange("p g i x -> p (g i x)"),
                    op=AOP.add)

                # per 512-sample sub-chunk: transposes + h1 accumulation
                h1_ps = None
                for sub in range(G1 // 4):
                    g0 = sub * 4
                    sidx = ch * (G1 // 4) + sub
                    col0 = sidx * 512
                    tz_ps = ps_t.tile([96, 2, 512], BF16, tag="tzps")
                    for t in range(4):
                        nc.tensor.transpose(
                            tz_ps[:, 0, t * 128:(t + 1) * 128],
                            ao_e[:, g0 + t].rearrange("p i d -> p (i d)"),
                            ident[:, :])
                        nc.tensor.transpose(
                            tz_ps[:, 1, t * 128:(t + 1) * 128],
                            z[:, g0 + t].rearrange("p i d -> p (i d)"),
                            ident[:, :])
                    tz = stag.tile([96, 2, 512], BF16, tag="tz_s")
                    nc.scalar.activation(
                        out=tz.rearrange("p k x -> p (k x)"),
                        in_=tz_ps.rearrange("p k x -> p (k x)"), func=AFT.Copy)

                    if sub % 2 == 0:
                        h1_ps = ps_h1.tile([128, 2, 512], F32, tag="h1ps")
                    hp = h1_ps[:, sub % 2]
                    nc.tensor.matmul(hp, c_mtab,
                                     mo[:, sub * 512:(sub + 1) * 512],
                                     start=True, stop=False)
                    nc.tensor.matmul(hp, c_dbT0, tz[:, 1],
                                     start=False, stop=False)
                    nc.tensor.matmul(hp, c_dbT1, tz[:, 0],
                                     start=False, stop=True)
                    if sub % 2 == 1:
                        sidx2 = sidx // 2
                        nc.scalar.activation(
                            out=h1[:, col0 - 512:col0 + 512],
                            in_=h1_ps.rearrange("p k x -> p (k x)"),
                            func=AFT.Copy, accum_out=s1p[:, sidx2:sidx2 + 1])
                        sq = sqp.tile([128, 1024], BF16, tag="sq")
                        nc.scalar.activation(
                            out=sq[...],
                            in_=h1_ps.rearrange("p k x -> p (k x)"),
                            func=AFT.Square,
                            accum_out=q1p[:, sidx2:sidx2 + 1])

        # counter plane0 cast-load: gated on early-P1 output via dummy copy
        nc.vector.tensor_copy(ost[0:1, 0, 0:1], h1[0:1, 0:1])
        nc.gpsimd.dma_start(out=ost[...], in_=ctv[:, 0])

        # --- BN finalize (per-core batch stats) ----------------------------
        def col_stats(parts, sp, qp, cc_name):
            sq01 = singles.tile([parts, 2], F32, tag=cc_name + "_sq01")
            nc.vector.tensor_reduce(out=sq01[:, 0:1], in_=sp,
                                    axis=mybir.AxisListType.X, op=AOP.add)
            nc.vector.tensor_reduce(out=sq01[:, 1:2], in_=qp,
                                    axis=mybir.AxisListType.X, op=AOP.add)
            return sq01[:, 0:1], sq01[:, 1:2]

        def bn_finalize(parts, s1g, q1g, g_col, b_col, cc_name, denom):
            mean = singles.tile([parts, 1], F32, tag=cc_name + "_mean")
            nc.vector.tensor_scalar_mul(mean[...], s1g, 1.0 / denom)
            msq = singles.tile([parts, 1], F32, tag=cc_name + "_msq")
            nc.vector.tensor_scalar_mul(msq[...], q1g, 1.0 / denom)
            m2 = singles.tile([parts, 1], F32, tag=cc_name + "_m2")
            nc.vector.tensor_tensor(out=m2[...], in0=mean[...], in1=mean[...],
                                    op=AOP.mult)
            var = singles.tile([parts, 1], F32, tag=cc_name + "_var")
            nc.vector.tensor_tensor(out=var[...], in0=msq[...], in1=m2[...],
                                    op=AOP.subtract)
            eps = singles.tile([parts, 1], F32, tag=cc_name + "_eps")
            nc.vector.memset(eps[...], 1e-5)
            std = singles.tile([parts, 1], F32, tag=cc_name + "_std")
            nc.scalar.activation(out=std[...], in_=var[...], func=AFT.Sqrt,
                                 bias=eps[...])
            rstd = singles.tile([parts, 1], F32, tag=cc_name + "_rstd")
            nc.vector.reciprocal(out=rstd[...], in_=std[...])
            scale = singles.tile([parts, 1], F32, tag=cc_name + "_scale")
            nc.vector.tensor_tensor(out=scale[...], in0=g_col[...],
                                    in1=rstd[...], op=AOP.mult)
            mscale = singles.tile([parts, 1], F32, tag=cc_name + "_ms")
            nc.vector.tensor_tensor(out=mscale[...], in0=mean[...],
                                    in1=scale[...], op=AOP.mult)
            nbias = singles.tile([parts, 1], F32, tag=cc_name + "_nb")
            nc.vector.tensor_tensor(out=nbias[...], in0=b_col[...],
                                    in1=mscale[...], op=AOP.subtract)
            return scale, nbias

        s1g, q1g = col_stats(128, s1p[:, :], q1p[:, :], "cc1")
        sc1, nb1 = bn_finalize(128, s1g, q1g, c_g1, c_b1, "cc1", float(b))

        # --- P2: fc2 -------------------------------------------------------
        with tc.tile_pool(name="ps_h2", bufs=2, space="PSUM") as ps_h2, \
             tc.tile_pool(name="h2st", bufs=2) as h2st:
            for sc2 in range(b // 2048):
                a1c = h2st.tile([128, 2048], BF16, tag="a1c")
                nc.scalar.activation(out=a1c[...],
                                     in_=h1[:, sc2 * 2048:(sc2 + 1) * 2048],
                                     func=AFT.Relu, bias=nb1[...],
                                     scale=sc1[...])
                h2_ps = ps_h2.tile([64, 4, 512], F32, tag="h2ps")
                for k in range(4):
                    nc.tensor.matmul(h2_ps[:, k], c_w2T,
                                     a1c[:, k * 512:(k + 1) * 512],
                                     start=True, stop=True)
                nc.scalar.activation(out=h2[:, sc2 * 2048:(sc2 + 1) * 2048],
                                     in_=h2_ps.rearrange("p k x -> p (k x)"),
                                     func=AFT.Copy,
                                     accum_out=s2p[:, sc2:sc2 + 1])
                sqv = h2st.tile([64, 2048], BF16, tag="sqv")
                nc.scalar.activation(out=sqv[...],
                                     in_=h2_ps.rearrange("p k x -> p (k x)"),
                                     func=AFT.Square,
                                     accum_out=q2p[:, sc2:sc2 + 1])

        s2g, q2g = col_stats(64, s2p[:, :], q2p[:, :], "cc2")
        sc2_, nb2_ = bn_finalize(64, s2g, q2g, c_g2, c_b2, "cc2", float(b))

        # --- P3: fc3 + counter add + writeback -----------------------------
        with tc.tile_pool(name="ps_o", bufs=4, space="PSUM") as ps_o, \
             tc.tile_pool(name="ctp3", bufs=3) as ctpool, \
             tc.tile_pool(name="a2p", bufs=2) as a2p, \
             tc.tile_pool(name="ost3", bufs=3) as ost3:
            out_r = out_t[:, :].rearrange("(g p) n -> p g n", p=128)
            for ch in range(NCH3):
                ctp1 = ctpool.tile([128, G3, NB], F8, tag="ctp1")
                nc.sync.dma_start(out=ctp1[...],
                                  in_=ctv[:, 1, ch * G3:(ch + 1) * G3])
                a2c = a2p.tile([65, CH3], BF16, tag="a2c")
                nc.sync.dma_start(out=a2c[64:65, :], in_=ones_in[:, 0:CH3])
                nc.scalar.activation(
                    out=a2c[0:64, :],
                    in_=h2[:, ch * CH3:(ch + 1) * CH3], func=AFT.Relu,
                    bias=nb2_[...], scale=sc2_[...])
                for g2 in range(G3 // 2):
                    o_ps = ps_o.tile([128, 2, NB], F32, tag="ops")
                    for k in range(2):
                        g = g2 * 2 + k
                        nc.tensor.matmul(o_ps[:, k],
                                         a2c[:, g * 128:(g + 1) * 128],
                                         c_w3aug, start=True, stop=False)
                        nc.tensor.matmul(o_ps[:, k], c_id8[:, :],
                                         ctp1[:, g],
                                         start=False, stop=True)
                    g0 = ch * G3 + g2 * 2
                    if g2 == 0:
                        # fused copy+add on DVE keeps ACT free for its copies
                        nc.vector.scalar_tensor_tensor(
                            out=ost[:, g0:g0 + 2],
                            in0=o_ps[...], scalar=1.0,
                            in1=ost[:, g0:g0 + 2],
                            op0=AOP.mult, op1=AOP.add)
                    else:
                        og = ost3.tile([128, 2, NB], BF16, tag="og")
                        nc.scalar.activation(
                            out=og.rearrange("p k x -> p (k x)"),
                            in_=o_ps.rearrange("p k x -> p (k x)"),
                            func=AFT.Copy)
                        nc.vector.tensor_tensor(
                            out=ost[:, g0:g0 + 2],
                            in0=og[...], in1=ost[:, g0:g0 + 2], op=AOP.add)
                nc.sync.dma_start(out=out_r[:, ch * G3:(ch + 1) * G3, :],
                                  in_=ost[:, ch * G3:(ch + 1) * G3, :])

    nc.compile()
    return nc


# ---------------------------------------------------------------------------
# host wrapper
# ---------------------------------------------------------------------------

_NC_CACHE = {}


def make_core_inputs(inputs, pc, b, n_cores):
    import ml_dtypes
    bf16 = ml_dtypes.bfloat16
    friends = np.asarray(inputs['friends'], np.int64)
    enemies = np.asarray(inputs['enemies'], np.int64)
    map_idx = np.asarray(inputs['map_idx'], np.int64)
    n = friends.shape[0]
    G1 = 16
    NCH1 = b // (G1 * 128)

    # bf16 tables: host gathers are then raw row copies
    TEA = pc['T_EA'].reshape(3 * NB, 3 * NB, NH).astype(bf16)
    TFA = pc['T_FA'].astype(bf16)
    TCA = pc['TCA'].astype(bf16)
    v_ea = pc['v_ea'].astype(bf16)
    cn = pc['cn'].astype(np.float32)
    P2f = cn[:, None, :] + cn[None, :, :]

    # per-sample record [S_ea | S_fa | Tt]
    rec = np.empty((n, REC), bf16)
    ei = np.arange(3)[None] * NB + enemies                  # (n, 3) token idx
    # scores stored (j_key, i_query, h): TEA[A,B][b,x,y] = s(query=y, key=x)
    rec[:, OFF_SEA:OFF_SFA] = TEA[ei[:, None, :], ei[:, :, None]].reshape(n, 36)
    rec[:, OFF_SFA:OFF_T] = TFA[friends[:, None, :], friends[:, :, None]] \
        .reshape(n, 36)
    la = np.concatenate([friends, np.full((n, 1), NB)], 1)
    mb = np.concatenate([ei, np.full((n, 1), 3 * NB)], 1)
    rec[:, OFF_T:REC] = TCA[la[:, :, None], mb[:, None, :]].reshape(n, 64)

    # value vectors, replicated over the query index: (n, 3j, 3i, 32)
    v_g = v_ea[np.arange(3)[None], enemies]                 # (n, 3, 32)
    vrep = np.repeat(v_g[:, :, None, :], 3, axis=2)         # (n, 3, 3, 32)

    f8 = ml_dtypes.float8_e4m3
    valid = (enemies != 0).sum(1)
    vi = np.maximum(valid, 1) - 1                       # 0..2 scale version
    P2s = np.empty((3, NB * NB, NB), f8)
    cns = np.empty((3, NB, NB), f8)
    for v in range(3):
        P2s[v] = (P2f * (1.0 / (v + 1))).reshape(NB * NB, NB).astype(f8)
        cns[v] = (cn * (1.0 / (v + 1))).astype(f8)
    ctg = np.empty((n, 2, NB), f8)
    ctg[:, 0] = P2s[vi, enemies[:, 0] * NB + enemies[:, 1]]
    ctg[:, 1] = cns[vi, enemies[:, 2]]

    eyeT = np.eye(NM, dtype=bf16)                       # one-hot source rows

    # packed constants
    cbf = np.zeros((128, 960), bf16)
    cbf[:, 0:128] = pc['m_tab'].astype(bf16)
    cbf[0:96, 128:384] = pc['dbT'].reshape(96, 256).astype(bf16)
    cbf[:, 384:448] = pc['w2T'].astype(bf16)
    cbf[0:65, 448:960] = pc['w3aug'].astype(bf16)
    cf = np.zeros((128, 4), np.float32)
    cf[:, 0] = pc['bn1_g']
    cf[:, 1] = pc['bn1_b']
    cf[0:64, 2] = pc['bn2_g']
    cf[0:64, 3] = pc['bn2_b']

    shared = dict(
        cbf=cbf, cf=cf,
        id8=np.eye(128).astype(f8),
        ones=np.ones((1, b), bf16),
    )

    in_maps = []
    for c in range(n_cores):
        lo, hi = c * b, (c + 1) * b
        m = dict(shared)
        m['xrec'] = np.ascontiguousarray(
            rec[lo:hi].reshape(b // 128, 128, REC).transpose(1, 0, 2)
        ).reshape(128, -1)
        # vr: (128, NCH1, 3j, G1, 3i*32)
        m['vr'] = np.ascontiguousarray(
            vrep[lo:hi].reshape(NCH1, G1, 128, 3, 96)
            .transpose(2, 0, 3, 1, 4)
        ).reshape(128, -1)
        # feature-major one-hot of map index: (128 maps, b)
        m['moh'] = np.ascontiguousarray(eyeT[map_idx[lo:hi, 0]].T)
        # counter rows, plane-major: (128, 2, b//128, 512)
        m['ctps'] = np.ascontiguousarray(
            ctg[lo:hi].reshape(b // 128, 128, 2, NB).transpose(1, 2, 0, 3)
        ).reshape(128, -1)
        in_maps.append(m)
    return in_maps


def kernel(**inputs):
    from concourse.bass_utils import run_bass_kernel_spmd
    b = B_FULL // NCORES
    pc = host_precompute(inputs)
    key = (b, NCORES)
    if key not in _NC_CACHE:
        _NC_CACHE[key] = build_nc(b, NCORES)
    nc = _NC_CACHE[key]
    in_maps = make_core_inputs(inputs, pc, b, NCORES)
    res = run_bass_kernel_spmd(nc, in_maps, core_ids=list(range(NCORES)))
    out = np.concatenate([np.asarray(r['out'], np.float32)
                          for r in res.results], 0)
    return out
